# revision 25
# baseline (speedup 1.0000x reference)
"""CAM (channel-attention) module kernel for Trainium2.

Computes, per batch b:
    q      = x[b].reshape(C, H*W)
    E      = q @ q.T                                  # [C, C] channel Gram matrix
    A[i,j] = softmax_j(rowmax_i(E) - E[i,j])          # suppression softmax
           = exp(rowmin_i(E) - E[i,j]) / Z_i
    out[b] = gamma * (A @ q) + x[b]

Distribution: pure data-parallel over batch B=16 across 8 NeuronCores
(2 batches per core); gamma replicated. No collectives.

Per-core kernel strategy (all matmuls on the PE; Gram in float32r at
1 cyc/row, attention-apply fully in fp16 with fast-weight-loads):
  1. q loaded natural-layout [128, 4, 4096] exact fp32 (the residual path
     needs the original bits). 512-col DMA chunks (2KB lines - narrower
     chunks transfer ~2x slower). ~32 dummy transposes of a VectorE-memset
     scratch ramp the PE out of the HAM 4/8 clock gate while the first
     chunk is in flight (scratch, not identity: the gpsimd-built identity
     isn't ready until ~8.5us, the memset by ~6us).
  2. qT built on-chip via PE transpose-mode in 128-column chunks,
     processed in PAIRS (8 transposes, then 8 Gram matmuls): transpose<->
     regular mode switches flush the PE pipeline, so longer same-mode
     runs pipeline better (measured 109 ns/transpose warm). The
     PSUM->SBUF copy is a rounding f32->f32r cast (walrus requires f32r
     operands from a rounding producer; fp32 matmul would be 4 cyc/row;
     16-bit Gram operands would push the suppression-softmax argmin
     precision past the error budget, and walrus rejects mixed
     f32r x fp16 matmuls).
  3. E computed block-upper-triangular only (symmetry); the strictly-lower
     128x128 blocks are mirrored with exact fp32 PE transposes (E feeds
     exp directly, so rounding there would be a real error).
  4. S = exp(rowmin - E) fused on ScalarE (bias=rowmin, scale=-1) writing
     fp16 directly, with accum_out producing Z = sum_j S in fp32. For the
     last batch, gamma/Z is then folded into S rows (ScalarE per-partition
     scale) so its epilogue is a bare residual add - shortens the kernel
     tail; the first batch keeps the scale in its (PE-overlapped) epilogue.
  5. S transposed 128x128-blockwise on PE in fp16 -> ST, jt-major with the
     SBUF copy fired per block so the next phase never waits on ST.
  6. U = ST.T @ qr fully in fp16 (qr = fp16 cast of a q column chunk on
     VectorE, cast 2 chunks ahead; fp16 weight loads hit the fast-weight-
     load path, 97 ns vs 190 ns for f32r, and hide under the 213 ns
     moving streams - measured 216 ns/matmul sustained). Epilogue adds the
     exact fp32 q and writes an fp16 out tile; one aggregated 1MB store
     per s-group (the last group splits store+epilogue to drain early).
  7. Output DRAM tensor is fp16 in an s-major layout [NCH, P, CT, FD] so
     each group store writes 4KB contiguous per partition (fp16 in the
     natural layout would drop to 1KB DMA lines at ~half rate); the host
     unscrambles and upcasts. Halves store HBM traffic and drain time;
     gamma=0 stays exact to fp16 rounding (4e-4).
  8. Cross-batch software pipelining: batch 1's transpose+Gram chunks are
     burst-interleaved with batch 0's attention-apply (4 s-groups in the
     Gram phase, 4 deferred into batch 1's softmax phase) so the PE never
     idles long enough for the HAM clock gate to re-throttle to 1.2 GHz.
"""

import sys

import numpy as np

if "/opt/trn_rl_repo" not in sys.path:
    sys.path.insert(0, "/opt/trn_rl_repo")

B, C, H, W = 16, 512, 64, 64
N = H * W                # 4096 spatial positions
P = 128                  # partitions
CT = C // P              # 4 channel tiles
KT = N // P              # 32 contraction chunks for the Gram matmul
FD = 512                 # matmul moving free dim / PSUM bank width (fp32)
NCH = N // FD            # 8 output column chunks
N_CORES = 8
BPC = B // N_CORES       # 2 batches per core

# Moving-operand start column for the upper-triangular Gram matmul. Row-tile 3
# widens from 128 to 256 columns: float32r only streams at 1 cyc/row when the
# output free dim is >= 256, so recomputing block (3,2) is cheaper than a
# 128-wide f32r matmul.
MVSTART = [0, 128, 256, 256]

_CACHE = {}


def _build_nc():
    from contextlib import ExitStack

    import concourse.bacc as bacc
    import concourse.tile as tile
    from concourse import mybir
    from concourse.masks import make_identity

    f32 = mybir.dt.float32
    f32r = mybir.dt.float32r
    f16 = mybir.dt.float16
    AF = mybir.ActivationFunctionType
    ALU = mybir.AluOpType

    nc = bacc.Bacc(None, target_bir_lowering=False)
    # x stays float32 end-to-end on the load path: the DMA cast unit ROUNDS
    # when the destination dtype is float32r, which would corrupt the
    # residual. Reduced-precision matmul operands are produced by engine
    # cast-copies (ScalarE/VectorE).
    x_d = nc.dram_tensor("x", [BPC, C, N], f32, kind="ExternalInput")
    g_d = nc.dram_tensor("gamma", [1], f32, kind="ExternalInput")
    # s-major out layout: each s-group store writes 4KB contiguous per
    # partition (fp16 needs >=2KB lines for full DMA rate; the natural
    # [C, N] layout would give 1KB lines). Host unscrambles.
    o_d = nc.dram_tensor(
        "out", [BPC, NCH, P, CT, FD], f16, kind="ExternalOutput"
    )

    with ExitStack() as ctx:
        tc = ctx.enter_context(tile.TileContext(nc))
        singles = ctx.enter_context(tc.tile_pool(name="singles", bufs=1))
        bigq = ctx.enter_context(tc.tile_pool(name="bigq", bufs=2))
        qtp = ctx.enter_context(tc.tile_pool(name="qtp", bufs=5))
        qrp = ctx.enter_context(tc.tile_pool(name="qrp", bufs=3))
        mats = ctx.enter_context(tc.tile_pool(name="mats", bufs=4))
        outp = ctx.enter_context(tc.tile_pool(name="outp", bufs=3))
        smallp = ctx.enter_context(tc.tile_pool(name="small", bufs=8))
        psp = ctx.enter_context(tc.tile_pool(name="ps", bufs=8, space="PSUM"))

        def ps_tile():
            return psp.tile([P, FD], f32, tag="ps", name="ps")

        LOOK = 2

        def emit_load(b, chunk=FD):
            xb = x_d[b].rearrange("(ct p) n -> p ct n", p=P)
            ob = o_d[b]
            q = bigq.tile([P, CT, N], f32, tag="q")
            # NB: 512-col-or-wider chunks only — narrower pieces drop the DMA
            # to sub-2KB lines, which transfer ~2x slower (measured). Batch 0
            # uses 512-col chunks (head latency); batch 1 coarser 1024-col.
            for s in range(N // chunk):
                nc.sync.dma_start(
                    out=q[:, :, s * chunk : (s + 1) * chunk],
                    in_=xb[:, :, s * chunk : (s + 1) * chunk],
                )
            return {"q": q, "xb": xb, "ob": ob}

        def emit_tr(st, k):
            q = st["q"]
            pst = psp.tile([P, FD], f32, tag="ps", name="pstr")
            for t in range(CT):
                nc.tensor.transpose(
                    pst[:, t * P : (t + 1) * P],
                    q[:, t, k * P : (k + 1) * P],
                    ident[:],
                )
            # rounding cast f32 -> f32r makes qk a legal f32r operand
            qk = qtp.tile([P, C], f32r, tag="qt")
            if k % 2 == 0:
                nc.scalar.copy(qk[:], pst[:])
            else:
                nc.vector.tensor_copy(qk[:], pst[:])
            st["qt"][k] = qk

        def emit_mm1(st, k):
            qkr = st["qt"][k]
            psE = st["psE"]
            for t in range(CT):
                w = C - MVSTART[t]
                nc.tensor.matmul(
                    psE[t][:, :w],
                    qkr[:, t * P : (t + 1) * P],
                    qkr[:, MVSTART[t] :],
                    start=(k == 0),
                    stop=(k == KT - 1),
                )

        def emit_cast(st, s):
            q = st["q"]
            qr = qrp.tile([P, CT, FD], f16, tag="qr")
            nc.vector.tensor_copy(qr[:], q[:, :, s * FD : (s + 1) * FD])
            st["qrs"][s] = qr

        def emit_mm2_s(st, s, last=False):
            # one s-chunk of mm2 + epilogue: 4 psU groups, 1 aggregated store
            if s == 0:
                emit_cast(st, 0)
                emit_cast(st, 1)
            if s + 2 < NCH:
                emit_cast(st, s + 2)
            qr = st["qrs"][s]
            q, ob, ST, grz = st["q"], st["ob"], st["ST"], st["grz"]
            ot = outp.tile([P, CT, FD], f16, tag="ot")
            for t in range(CT):
                pu = ps_tile()
                for jt in range(CT):
                    nc.tensor.matmul(
                        pu[:],
                        ST[jt][:, t * P : (t + 1) * P],
                        qr[:, jt, :],
                        start=(jt == 0),
                        stop=(jt == CT - 1),
                    )
                if st["scaled"]:
                    # gamma/Z folded into S: bare residual add
                    nc.vector.tensor_add(
                        ot[:, t, :], pu[:], q[:, t, s * FD : (s + 1) * FD]
                    )
                elif t < 2:
                    nc.scalar.mul(pu[:], pu[:], grz[t][:])
                    nc.vector.tensor_add(
                        ot[:, t, :], pu[:], q[:, t, s * FD : (s + 1) * FD]
                    )
                else:
                    nc.vector.scalar_tensor_tensor(
                        ot[:, t, :],
                        pu[:],
                        grz[t][:],
                        q[:, t, s * FD : (s + 1) * FD],
                        op0=ALU.mult,
                        op1=ALU.add,
                    )
                if last and t == 1:
                    # half-store so the final drain overlaps tiles 2-3
                    nc.sync.dma_start(out=ob[s, :, 0:2, :], in_=ot[:, 0:2, :])
            if last:
                nc.sync.dma_start(out=ob[s, :, 2:CT, :], in_=ot[:, 2:CT, :])
            else:
                nc.sync.dma_start(out=ob[s], in_=ot[:])

        def emit_gram(st, prev, skip_chunks=0):
            """Transposes + Gram matmul for `st`, burst-interleaved with the
            previous batch's attention-apply (mm2) so PE never idles long
            enough for the HAM clock gate to re-throttle."""
            st["psE"] = [ps_tile() for _ in range(CT)]
            if "qt" not in st:
                st["qt"] = [None] * KT
            # chunks below skip_chunks had their transposes emitted during the
            # previous batch's softmax phase; emit their (lookahead-deferred)
            # Gram matmuls now so the accumulation starts with chunk 0
            for kk in range(max(0, skip_chunks - LOOK)):
                emit_mm1(st, kk)
            # chunks processed in groups (GRP x4 transposes, then GRP x4
            # Gram matmuls): transpose-mode <-> regular-mode switches flush
            # the PE pipeline, so longer same-mode runs pipeline better.
            # The solo first-batch phase uses quads (4 spare PSUM banks);
            # interleaved phases use pairs (mm2 groups need banks too).
            grp = 4 if prev is None and skip_chunks == 0 else 2
            for k in range(skip_chunks, KT, grp):
                for kk in range(k, k + grp):
                    emit_tr(st, kk)
                if k >= grp:
                    for kk in range(k - grp, k):
                        emit_mm1(st, kk)
                # only 4 of 8 s-groups here: the other four fill this batch's
                # own softmax phase, where the PE would otherwise idle
                if (
                    prev is not None
                    and k >= 6
                    and (k - 6) % 4 == 0
                    and (k - 6) // 4 < NCH - 4
                ):
                    emit_mm2_s(prev, (k - 6) // 4)
            for k in range(KT - grp, KT):
                emit_mm1(st, k)

        def emit_softmax(st, prev=None, scale_s=False):
            # ---- copy E out of PSUM; mirror strictly-lower blocks ----
            psE = st["psE"]
            E = []
            for t in range(CT):
                e = mats.tile([P, FD], f32, tag="E")
                w = C - MVSTART[t]
                if t % 2 == 0:
                    nc.scalar.copy(e[:, MVSTART[t] :], psE[t][:, :w])
                else:
                    nc.vector.tensor_copy(e[:, MVSTART[t] :], psE[t][:, :w])
                E.append(e)
            # E[t][:, s-block] = E[s][:, t-block].T for s < t (exact fp32
            # transposes: E magnitudes are ~4e3 and feed exp directly, so
            # low-precision rounding here would be a real error).
            for t in range(1, CT):
                for s in range(t):
                    if t == 3 and s == 2:
                        continue  # computed directly via the widened row-tile 3
                    pm = ps_tile()
                    nc.tensor.transpose(
                        pm[:, :P], E[s][:, t * P : (t + 1) * P], ident[:]
                    )
                    if (t + s) % 2 == 0:
                        nc.scalar.copy(E[t][:, s * P : (s + 1) * P], pm[:, :P])
                    else:
                        nc.vector.tensor_copy(
                            E[t][:, s * P : (s + 1) * P], pm[:, :P]
                        )

            # deferred mm2 s-group of the previous batch keeps the PE busy
            # while the rowmin/exp chains run on VectorE/ScalarE; for the
            # first batch, the NEXT batch's first transposes fill in instead
            if prev is not None:
                emit_mm2_s(prev, NCH - 4)
            elif st.get("next") is not None:
                emit_tr(st["next"], 0)
                emit_tr(st["next"], 1)

            # ---- suppression softmax: S = exp(rowmin - E), Z = rowsum(S),
            # S written as fp16 (legal fast-weight-load transpose operand) ----
            S = []
            grz = []
            for t in range(CT):
                rm = smallp.tile([P, 1], f32, tag="rm")
                nc.vector.tensor_reduce(
                    rm[:], E[t][:], axis=mybir.AxisListType.X, op=ALU.min
                )
                s_t = mats.tile([P, FD], f16, tag="S")
                z = smallp.tile([P, 1], f32, tag="z")
                nc.scalar.activation(
                    s_t[:], E[t][:], AF.Exp, bias=rm[:], scale=-1.0, accum_out=z[:]
                )
                rz = smallp.tile([P, 1], f32, tag="rz")
                nc.vector.reciprocal(rz[:], z[:])
                g = smallp.tile([P, 1], f32, tag="grz")
                nc.vector.tensor_mul(g[:], rz[:], gam[:])
                if scale_s:
                    # fold gamma/Z into S itself (per-partition row scale):
                    # the attention-apply output is then pre-scaled and the
                    # epilogue is a bare residual add. Only done where the
                    # softmax phase has deferred-mm2 PE cover for the longer
                    # chain (the last batch); the first batch keeps the
                    # scale in its epilogue.
                    nc.scalar.mul(s_t[:], s_t[:], g[:])
                S.append(s_t)
                grz.append(g)

            if prev is not None:
                emit_mm2_s(prev, NCH - 3)
            elif st.get("next") is not None:
                emit_tr(st["next"], 2)
                emit_tr(st["next"], 3)

            # ---- ST = S.T (attention^T), 128x128 fp16 blocks on PE ----
            # jt-major: each pstS[jt] completes after 4 transposes so its
            # SBUF copy overlaps the next jt's transposes; the final deferred
            # mm2 group runs after, covering the last copies' drain so the
            # next phase's first matmul never waits on ST
            ST = []
            for jt in range(CT):
                pstS = psp.tile([P, FD], f16, tag="ps", name="pstS")
                for t in range(CT):
                    nc.tensor.transpose(
                        pstS[:, t * P : (t + 1) * P],
                        S[t][:, jt * P : (jt + 1) * P],
                        identh[:],
                    )
                stj = mats.tile([P, FD], f16, tag="ST")
                if jt % 2 == 0:
                    nc.scalar.copy(stj[:], pstS[:])
                else:
                    nc.vector.tensor_copy(stj[:], pstS[:])
                ST.append(stj)
            if prev is not None:
                emit_mm2_s(prev, NCH - 2)
                emit_mm2_s(prev, NCH - 1)
            st["ST"] = ST
            st["scaled"] = scale_s
            st["grz"] = grz
            st["qrs"] = [None] * NCH

        # ---- pipelined driver: batch b's Gram phase overlaps batch b-1's
        # attention-apply phase on the PE ----
        st0 = emit_load(0)
        st1 = emit_load(1, chunk=2 * FD)

        # dummy transposes ramp the PE while the first load chunk is still
        # in flight, so the HAM clock gate reaches 8/8 before real work
        # starts (a cold PE runs ~3.4us at half clock otherwise). The
        # scratch operand comes from a VectorE memset so warm-up starts
        # ~3us before the (gpsimd-built) identity is ready.
        wsrc = singles.tile([P, P], f32)
        nc.vector.memset(wsrc[:], 0.0)
        for _ in range(8):
            pw = psp.tile([P, P], f32, tag="ps", name="warm")
            for _r in range(4):
                nc.tensor.transpose(pw[:], wsrc[:], wsrc[:])

        ident = singles.tile([P, P], f32)
        make_identity(nc, ident)
        identh = singles.tile([P, P], f16)
        nc.vector.tensor_copy(identh[:], ident[:])

        # gamma broadcast to all partitions as a per-partition scalar
        gam = singles.tile([P, 1], f32)
        nc.gpsimd.dma_start(out=gam[:], in_=g_d[:].to_broadcast([P, 1]))

        emit_gram(st0, None)
        st1["qt"] = [None] * KT
        st0["next"] = st1
        emit_softmax(st0, None)
        emit_gram(st1, st0, skip_chunks=4)
        emit_softmax(st1, st0, scale_s=True)
        for s in range(NCH):
            emit_mm2_s(st1, s, last=(s == NCH - 1))

    nc.compile()
    return nc


def _get_nc():
    if "nc" not in _CACHE:
        _CACHE["nc"] = _build_nc()
    return _CACHE["nc"]


def kernel(x: np.ndarray, gamma: np.ndarray) -> np.ndarray:
    from concourse.bass_utils import run_bass_kernel_spmd

    nc = _get_nc()
    x = np.ascontiguousarray(np.asarray(x, dtype=np.float32))
    gamma = np.ascontiguousarray(np.asarray(gamma, dtype=np.float32))
    xs = x.reshape(B, C, N)
    in_maps = [
        {
            "x": np.ascontiguousarray(xs[c * BPC : (c + 1) * BPC]),
            "gamma": gamma,
        }
        for c in range(N_CORES)
    ]
    res = run_bass_kernel_spmd(nc, in_maps, core_ids=list(range(N_CORES)))
    out = np.stack(
        [np.asarray(res.results[c]["out"]) for c in range(N_CORES)], axis=0
    )
    # [cores, BPC, NCH, P, CT, FD] -> [B, C=(ct*128+p), N=(s*512+f)]
    out = out.reshape(N_CORES * BPC, NCH, P, CT, FD)
    out = out.transpose(0, 3, 2, 1, 4).reshape(B, C, N)
    return out.reshape(B, C, H, W).astype(np.float32)


# revision 26
# speedup vs baseline: 1.1915x; 1.1915x over previous
"""CAM (channel-attention) module kernel for Trainium2.

Computes, per batch b:
    q      = x[b].reshape(C, H*W)
    E      = q @ q.T                                  # [C, C] channel Gram matrix
    A[i,j] = softmax_j(rowmax_i(E) - E[i,j])          # suppression softmax
           = exp(rowmin_i(E) - E[i,j]) / Z_i
    out[b] = gamma * (A @ q) + x[b]

Distribution: pure data-parallel over batch B=16 across 8 NeuronCores
(2 batches per core); gamma replicated. No collectives.

Per-core kernel strategy (all matmuls on the PE; Gram in float32r at
1 cyc/row, attention-apply fully in fp16 with fast-weight-loads):
  1. q loaded natural-layout [128, 4, 4096] exact fp32 (the residual path
     needs the original bits). 512-col DMA chunks (2KB lines - narrower
     chunks transfer ~2x slower). ~32 dummy transposes of a VectorE-memset
     scratch ramp the PE out of the HAM 4/8 clock gate while the first
     chunk is in flight (scratch, not identity: the gpsimd-built identity
     isn't ready until ~8.5us, the memset by ~6us).
  2. qT built on-chip via PE transpose-mode in 128-column chunks,
     processed in PAIRS (8 transposes, then 8 Gram matmuls): transpose<->
     regular mode switches flush the PE pipeline, so longer same-mode
     runs pipeline better (measured 109 ns/transpose warm). The
     PSUM->SBUF copy is a rounding f32->f32r cast (walrus requires f32r
     operands from a rounding producer; fp32 matmul would be 4 cyc/row;
     16-bit Gram operands would push the suppression-softmax argmin
     precision past the error budget, and walrus rejects mixed
     f32r x fp16 matmuls).
  3. E computed block-upper-triangular only (symmetry); the strictly-lower
     128x128 blocks are mirrored with exact fp32 PE transposes (E feeds
     exp directly, so rounding there would be a real error).
  4. S = exp(rowmin - E) fused on ScalarE (bias=rowmin, scale=-1) writing
     fp16 directly, with accum_out producing Z = sum_j S in fp32. For the
     last batch, gamma/Z is then folded into S rows (ScalarE per-partition
     scale) so its epilogue is a bare residual add - shortens the kernel
     tail; the first batch keeps the scale in its (PE-overlapped) epilogue.
  5. S transposed 128x128-blockwise on PE in fp16 -> ST, jt-major with the
     SBUF copy fired per block so the next phase never waits on ST.
  6. U = ST.T @ qr fully in fp16 (qr = fp16 cast of a q column chunk on
     VectorE, cast 2 chunks ahead; fp16 weight loads hit the fast-weight-
     load path, 97 ns vs 190 ns for f32r, and hide under the 213 ns
     moving streams - measured 216 ns/matmul sustained). Epilogue adds the
     exact fp32 q and writes an fp16 out tile; one aggregated 1MB store
     per s-group (the last group splits store+epilogue to drain early).
  7. Output DRAM tensor is fp16 in an s-major layout [NCH, P, CT, FD] so
     each group store writes 4KB contiguous per partition (fp16 in the
     natural layout would drop to 1KB DMA lines at ~half rate); the host
     unscrambles and upcasts. Halves store HBM traffic and drain time;
     gamma=0 stays exact to fp16 rounding (4e-4).
  8. Cross-batch software pipelining: batch 1's transpose+Gram chunks are
     burst-interleaved with batch 0's attention-apply (4 s-groups in the
     Gram phase, 4 deferred into batch 1's softmax phase) so the PE never
     idles long enough for the HAM clock gate to re-throttle to 1.2 GHz.
"""

import sys

import numpy as np

if "/opt/trn_rl_repo" not in sys.path:
    sys.path.insert(0, "/opt/trn_rl_repo")

B, C, H, W = 16, 512, 64, 64
N = H * W                # 4096 spatial positions
P = 128                  # partitions
CT = C // P              # 4 channel tiles
KT = N // P              # 32 contraction chunks for the Gram matmul
FD = 512                 # matmul moving free dim / PSUM bank width (fp32)
NCH = N // FD            # 8 output column chunks
N_CORES = 8
BPC = B // N_CORES       # 2 batches per core

# Moving-operand start column for the upper-triangular Gram matmul. Row-tile 3
# widens from 128 to 256 columns: float32r only streams at 1 cyc/row when the
# output free dim is >= 256, so recomputing block (3,2) is cheaper than a
# 128-wide f32r matmul.
MVSTART = [0, 128, 256, 256]

_CACHE = {}


def _build_nc():
    from contextlib import ExitStack

    import concourse.bacc as bacc
    import concourse.tile as tile
    from concourse import mybir
    from concourse.masks import make_identity

    f32 = mybir.dt.float32
    f32r = mybir.dt.float32r
    f16 = mybir.dt.float16
    AF = mybir.ActivationFunctionType
    ALU = mybir.AluOpType

    nc = bacc.Bacc(None, target_bir_lowering=False)
    # x stays float32 end-to-end on the load path: the DMA cast unit ROUNDS
    # when the destination dtype is float32r, which would corrupt the
    # residual. Reduced-precision matmul operands are produced by engine
    # cast-copies (ScalarE/VectorE).
    x_d = nc.dram_tensor("x", [BPC, C, N], f32, kind="ExternalInput")
    g_d = nc.dram_tensor("gamma", [1], f32, kind="ExternalInput")
    # s-major out layout: each s-group store writes 4KB contiguous per
    # partition (fp16 needs >=2KB lines for full DMA rate; the natural
    # [C, N] layout would give 1KB lines). Host unscrambles.
    o_d = nc.dram_tensor(
        "out", [BPC, NCH, P, CT, FD], f16, kind="ExternalOutput"
    )

    with ExitStack() as ctx:
        tc = ctx.enter_context(tile.TileContext(nc))
        singles = ctx.enter_context(tc.tile_pool(name="singles", bufs=1))
        bigq = ctx.enter_context(tc.tile_pool(name="bigq", bufs=2))
        qtp = ctx.enter_context(tc.tile_pool(name="qtp", bufs=5))
        qrp = ctx.enter_context(tc.tile_pool(name="qrp", bufs=3))
        mats = ctx.enter_context(tc.tile_pool(name="mats", bufs=4))
        outp = ctx.enter_context(tc.tile_pool(name="outp", bufs=3))
        smallp = ctx.enter_context(tc.tile_pool(name="small", bufs=8))
        psp = ctx.enter_context(tc.tile_pool(name="ps", bufs=8, space="PSUM"))

        def ps_tile():
            return psp.tile([P, FD], f32, tag="ps", name="ps")

        LOOK = 2

        def emit_load(b, chunk=FD):
            xb = x_d[b].rearrange("(ct p) n -> p ct n", p=P)
            ob = o_d[b]
            q = bigq.tile([P, CT, N], f32, tag="q")
            # NB: 512-col-or-wider chunks only — narrower pieces drop the DMA
            # to sub-2KB lines, which transfer ~2x slower (measured). Batch 0
            # uses 512-col chunks (head latency); batch 1 coarser 1024-col.
            for s in range(N // chunk):
                nc.sync.dma_start(
                    out=q[:, :, s * chunk : (s + 1) * chunk],
                    in_=xb[:, :, s * chunk : (s + 1) * chunk],
                )
            return {"q": q, "xb": xb, "ob": ob}

        def emit_tr(st, k):
            q = st["q"]
            pst = psp.tile([P, FD], f32, tag="ps", name="pstr")
            for t in range(CT):
                nc.tensor.transpose(
                    pst[:, t * P : (t + 1) * P],
                    q[:, t, k * P : (k + 1) * P],
                    ident[:],
                )
            # rounding cast f32 -> f32r makes qk a legal f32r operand
            qk = qtp.tile([P, C], f32r, tag="qt")
            if k % 2 == 0:
                nc.scalar.copy(qk[:], pst[:])
            else:
                nc.vector.tensor_copy(qk[:], pst[:])
            st["qt"][k] = qk

        def emit_mm1(st, k):
            qkr = st["qt"][k]
            psE = st["psE"]
            for t in range(CT):
                w = C - MVSTART[t]
                nc.tensor.matmul(
                    psE[t][:, :w],
                    qkr[:, t * P : (t + 1) * P],
                    qkr[:, MVSTART[t] :],
                    start=(k == 0),
                    stop=(k == KT - 1),
                )

        def emit_cast(st, s):
            q = st["q"]
            qr = qrp.tile([P, CT, FD], f16, tag="qr")
            nc.vector.tensor_copy(qr[:], q[:, :, s * FD : (s + 1) * FD])
            st["qrs"][s] = qr

        def emit_mm2_s(st, s, last=False):
            # one s-chunk of mm2 + epilogue: 4 psU groups, 1 aggregated store
            if s == 0:
                emit_cast(st, 0)
                emit_cast(st, 1)
            if s + 2 < NCH:
                emit_cast(st, s + 2)
            qr = st["qrs"][s]
            q, ob, ST, grz = st["q"], st["ob"], st["ST"], st["grz"]
            ot = outp.tile([P, CT, FD], f16, tag="ot")
            for t in range(CT):
                pu = ps_tile()
                for jt in range(CT):
                    nc.tensor.matmul(
                        pu[:],
                        ST[jt][:, t * P : (t + 1) * P],
                        qr[:, jt, :],
                        start=(jt == 0),
                        stop=(jt == CT - 1),
                    )
                if st["scaled"]:
                    # gamma/Z folded into S: bare residual add
                    nc.vector.tensor_add(
                        ot[:, t, :], pu[:], q[:, t, s * FD : (s + 1) * FD]
                    )
                elif t < 2:
                    nc.scalar.mul(pu[:], pu[:], grz[t][:])
                    nc.vector.tensor_add(
                        ot[:, t, :], pu[:], q[:, t, s * FD : (s + 1) * FD]
                    )
                else:
                    nc.vector.scalar_tensor_tensor(
                        ot[:, t, :],
                        pu[:],
                        grz[t][:],
                        q[:, t, s * FD : (s + 1) * FD],
                        op0=ALU.mult,
                        op1=ALU.add,
                    )
                if last and t == 1:
                    # half-store so the final drain overlaps tiles 2-3
                    nc.sync.dma_start(out=ob[s, :, 0:2, :], in_=ot[:, 0:2, :])
            if last:
                nc.sync.dma_start(out=ob[s, :, 2:CT, :], in_=ot[:, 2:CT, :])
            else:
                nc.sync.dma_start(out=ob[s], in_=ot[:])

        def emit_gram(st, prev, skip_chunks=0):
            """Transposes + Gram matmul for `st`, burst-interleaved with the
            previous batch's attention-apply (mm2) so PE never idles long
            enough for the HAM clock gate to re-throttle."""
            st["psE"] = [ps_tile() for _ in range(CT)]
            if "qt" not in st:
                st["qt"] = [None] * KT
            # chunks below skip_chunks had their transposes emitted during the
            # previous batch's softmax phase; emit their (lookahead-deferred)
            # Gram matmuls now so the accumulation starts with chunk 0
            for kk in range(max(0, skip_chunks - LOOK)):
                emit_mm1(st, kk)
            # chunks processed in groups (GRP x4 transposes, then GRP x4
            # Gram matmuls): transpose-mode <-> regular-mode switches flush
            # the PE pipeline, so longer same-mode runs pipeline better.
            # Pairs everywhere: quads were tried for the solo first-batch
            # phase (4 spare PSUM banks) and measured ~5% WORSE - with all
            # 8 banks live the pool serializes on bank recycling.
            grp = 2
            for k in range(skip_chunks, KT, grp):
                for kk in range(k, k + grp):
                    emit_tr(st, kk)
                if k >= grp:
                    for kk in range(k - grp, k):
                        emit_mm1(st, kk)
                # only 4 of 8 s-groups here: the other four fill this batch's
                # own softmax phase, where the PE would otherwise idle
                if (
                    prev is not None
                    and k >= 6
                    and (k - 6) % 4 == 0
                    and (k - 6) // 4 < NCH - 4
                ):
                    emit_mm2_s(prev, (k - 6) // 4)
            for k in range(KT - grp, KT):
                emit_mm1(st, k)

        def emit_softmax(st, prev=None, scale_s=False):
            # ---- copy E out of PSUM; mirror strictly-lower blocks ----
            psE = st["psE"]
            E = []
            for t in range(CT):
                e = mats.tile([P, FD], f32, tag="E")
                w = C - MVSTART[t]
                if t % 2 == 0:
                    nc.scalar.copy(e[:, MVSTART[t] :], psE[t][:, :w])
                else:
                    nc.vector.tensor_copy(e[:, MVSTART[t] :], psE[t][:, :w])
                E.append(e)
            # E[t][:, s-block] = E[s][:, t-block].T for s < t (exact fp32
            # transposes: E magnitudes are ~4e3 and feed exp directly, so
            # low-precision rounding here would be a real error).
            for t in range(1, CT):
                for s in range(t):
                    if t == 3 and s == 2:
                        continue  # computed directly via the widened row-tile 3
                    pm = ps_tile()
                    nc.tensor.transpose(
                        pm[:, :P], E[s][:, t * P : (t + 1) * P], ident[:]
                    )
                    if (t + s) % 2 == 0:
                        nc.scalar.copy(E[t][:, s * P : (s + 1) * P], pm[:, :P])
                    else:
                        nc.vector.tensor_copy(
                            E[t][:, s * P : (s + 1) * P], pm[:, :P]
                        )

            # deferred mm2 s-group of the previous batch keeps the PE busy
            # while the rowmin/exp chains run on VectorE/ScalarE; for the
            # first batch, the NEXT batch's first transposes fill in instead
            if prev is not None:
                emit_mm2_s(prev, NCH - 4)
            elif st.get("next") is not None:
                emit_tr(st["next"], 0)
                emit_tr(st["next"], 1)

            # ---- suppression softmax: S = exp(rowmin - E), Z = rowsum(S),
            # S written as fp16 (legal fast-weight-load transpose operand) ----
            S = []
            grz = []
            for t in range(CT):
                rm = smallp.tile([P, 1], f32, tag="rm")
                nc.vector.tensor_reduce(
                    rm[:], E[t][:], axis=mybir.AxisListType.X, op=ALU.min
                )
                s_t = mats.tile([P, FD], f16, tag="S")
                z = smallp.tile([P, 1], f32, tag="z")
                nc.scalar.activation(
                    s_t[:], E[t][:], AF.Exp, bias=rm[:], scale=-1.0, accum_out=z[:]
                )
                rz = smallp.tile([P, 1], f32, tag="rz")
                nc.vector.reciprocal(rz[:], z[:])
                g = smallp.tile([P, 1], f32, tag="grz")
                nc.vector.tensor_mul(g[:], rz[:], gam[:])
                if scale_s:
                    # fold gamma/Z into S itself (per-partition row scale):
                    # the attention-apply output is then pre-scaled and the
                    # epilogue is a bare residual add. Only done where the
                    # softmax phase has deferred-mm2 PE cover for the longer
                    # chain (the last batch); the first batch keeps the
                    # scale in its epilogue.
                    nc.scalar.mul(s_t[:], s_t[:], g[:])
                S.append(s_t)
                grz.append(g)

            if prev is not None:
                emit_mm2_s(prev, NCH - 3)
            elif st.get("next") is not None:
                emit_tr(st["next"], 2)
                emit_tr(st["next"], 3)

            # ---- ST = S.T (attention^T), 128x128 fp16 blocks on PE ----
            # jt-major: each pstS[jt] completes after 4 transposes so its
            # SBUF copy overlaps the next jt's transposes; the final deferred
            # mm2 group runs after, covering the last copies' drain so the
            # next phase's first matmul never waits on ST
            ST = []
            for jt in range(CT):
                pstS = psp.tile([P, FD], f16, tag="ps", name="pstS")
                for t in range(CT):
                    nc.tensor.transpose(
                        pstS[:, t * P : (t + 1) * P],
                        S[t][:, jt * P : (jt + 1) * P],
                        identh[:],
                    )
                stj = mats.tile([P, FD], f16, tag="ST")
                if jt % 2 == 0:
                    nc.scalar.copy(stj[:], pstS[:])
                else:
                    nc.vector.tensor_copy(stj[:], pstS[:])
                ST.append(stj)
            if prev is not None:
                emit_mm2_s(prev, NCH - 2)
                emit_mm2_s(prev, NCH - 1)
            st["ST"] = ST
            st["scaled"] = scale_s
            st["grz"] = grz
            st["qrs"] = [None] * NCH

        # ---- pipelined driver: batch b's Gram phase overlaps batch b-1's
        # attention-apply phase on the PE ----
        st0 = emit_load(0)
        st1 = emit_load(1, chunk=2 * FD)

        # dummy transposes ramp the PE while the first load chunk is still
        # in flight, so the HAM clock gate reaches 8/8 before real work
        # starts (a cold PE runs ~3.4us at half clock otherwise). The
        # scratch operand comes from a VectorE memset so warm-up starts
        # ~3us before the (gpsimd-built) identity is ready.
        wsrc = singles.tile([P, P], f32)
        nc.vector.memset(wsrc[:], 0.0)
        for _ in range(8):
            pw = psp.tile([P, P], f32, tag="ps", name="warm")
            for _r in range(4):
                nc.tensor.transpose(pw[:], wsrc[:], wsrc[:])

        ident = singles.tile([P, P], f32)
        make_identity(nc, ident)
        identh = singles.tile([P, P], f16)
        nc.vector.tensor_copy(identh[:], ident[:])

        # gamma broadcast to all partitions as a per-partition scalar
        gam = singles.tile([P, 1], f32)
        nc.gpsimd.dma_start(out=gam[:], in_=g_d[:].to_broadcast([P, 1]))

        emit_gram(st0, None)
        st1["qt"] = [None] * KT
        st0["next"] = st1
        emit_softmax(st0, None)
        emit_gram(st1, st0, skip_chunks=4)
        emit_softmax(st1, st0, scale_s=True)
        for s in range(NCH):
            emit_mm2_s(st1, s, last=(s == NCH - 1))

    nc.compile()
    return nc


def _get_nc():
    if "nc" not in _CACHE:
        _CACHE["nc"] = _build_nc()
    return _CACHE["nc"]


def kernel(x: np.ndarray, gamma: np.ndarray) -> np.ndarray:
    from concourse.bass_utils import run_bass_kernel_spmd

    nc = _get_nc()
    x = np.ascontiguousarray(np.asarray(x, dtype=np.float32))
    gamma = np.ascontiguousarray(np.asarray(gamma, dtype=np.float32))
    xs = x.reshape(B, C, N)
    in_maps = [
        {
            "x": np.ascontiguousarray(xs[c * BPC : (c + 1) * BPC]),
            "gamma": gamma,
        }
        for c in range(N_CORES)
    ]
    res = run_bass_kernel_spmd(nc, in_maps, core_ids=list(range(N_CORES)))
    out = np.stack(
        [np.asarray(res.results[c]["out"]) for c in range(N_CORES)], axis=0
    )
    # [cores, BPC, NCH, P, CT, FD] -> [B, C=(ct*128+p), N=(s*512+f)]
    out = out.reshape(N_CORES * BPC, NCH, P, CT, FD)
    out = out.transpose(0, 3, 2, 1, 4).reshape(B, C, N)
    return out.reshape(B, C, H, W).astype(np.float32)


# revision 27
# speedup vs baseline: 1.1974x; 1.0050x over previous
"""CAM (channel-attention) module kernel for Trainium2.

Computes, per batch b:
    q      = x[b].reshape(C, H*W)
    E      = q @ q.T                                  # [C, C] channel Gram matrix
    A[i,j] = softmax_j(rowmax_i(E) - E[i,j])          # suppression softmax
           = exp(rowmin_i(E) - E[i,j]) / Z_i
    out[b] = gamma * (A @ q) + x[b]

Distribution: pure data-parallel over batch B=16 across 8 NeuronCores
(2 batches per core); gamma replicated. No collectives.

Per-core kernel strategy (all matmuls on the PE; Gram in float32r at
1 cyc/row, attention-apply fully in fp16 with fast-weight-loads):
  1. q loaded natural-layout [128, 4, 4096] exact fp32 (the residual path
     needs the original bits). 512-col DMA chunks (2KB lines - narrower
     chunks transfer ~2x slower). ~32 dummy transposes of a VectorE-memset
     scratch ramp the PE out of the HAM 4/8 clock gate while the first
     chunk is in flight (scratch, not identity: the gpsimd-built identity
     isn't ready until ~8.5us, the memset by ~6us).
  2. qT built on-chip via PE transpose-mode in 128-column chunks,
     processed in PAIRS (8 transposes, then 8 Gram matmuls): transpose<->
     regular mode switches flush the PE pipeline, so longer same-mode
     runs pipeline better (measured 109 ns/transpose warm). The
     PSUM->SBUF copy is a rounding f32->f32r cast (walrus requires f32r
     operands from a rounding producer; fp32 matmul would be 4 cyc/row;
     16-bit Gram operands would push the suppression-softmax argmin
     precision past the error budget, and walrus rejects mixed
     f32r x fp16 matmuls).
  3. E computed block-upper-triangular only (symmetry); the strictly-lower
     128x128 blocks are mirrored with exact fp32 PE transposes (E feeds
     exp directly, so rounding there would be a real error).
  4. S = exp(rowmin - E) fused on ScalarE (bias=rowmin, scale=-1) writing
     fp16 directly, with accum_out producing Z = sum_j S in fp32. For the
     last batch, gamma/Z is then folded into S rows (ScalarE per-partition
     scale) so its epilogue is a bare residual add - shortens the kernel
     tail; the first batch keeps the scale in its (PE-overlapped) epilogue.
  5. S transposed 128x128-blockwise on PE in fp16 -> ST, jt-major with the
     SBUF copy fired per block so the next phase never waits on ST.
  6. U = ST.T @ qr fully in fp16 (qr = fp16 cast of a q column chunk on
     VectorE, cast 2 chunks ahead; fp16 weight loads hit the fast-weight-
     load path, 97 ns vs 190 ns for f32r, and hide under the 213 ns
     moving streams - measured 216 ns/matmul sustained). Epilogue adds the
     exact fp32 q and writes an fp16 out tile; one aggregated 1MB store
     per s-group (the last group splits store+epilogue to drain early).
  7. Output DRAM tensor is fp16 in an s-major layout [NCH, P, CT, FD] so
     each group store writes 4KB contiguous per partition (fp16 in the
     natural layout would drop to 1KB DMA lines at ~half rate); the host
     unscrambles and upcasts. Halves store HBM traffic and drain time;
     gamma=0 stays exact to fp16 rounding (4e-4).
  8. Cross-batch software pipelining: batch 1's transpose+Gram chunks are
     burst-interleaved with batch 0's attention-apply (4 s-groups in the
     Gram phase, 4 deferred into batch 1's softmax phase) so the PE never
     idles long enough for the HAM clock gate to re-throttle to 1.2 GHz.
"""

import sys

import numpy as np

if "/opt/trn_rl_repo" not in sys.path:
    sys.path.insert(0, "/opt/trn_rl_repo")

B, C, H, W = 16, 512, 64, 64
N = H * W                # 4096 spatial positions
P = 128                  # partitions
CT = C // P              # 4 channel tiles
KT = N // P              # 32 contraction chunks for the Gram matmul
FD = 512                 # matmul moving free dim / PSUM bank width (fp32)
NCH = N // FD            # 8 output column chunks
N_CORES = 8
BPC = B // N_CORES       # 2 batches per core

# Moving-operand start column for the upper-triangular Gram matmul. Row-tile 3
# widens from 128 to 256 columns: float32r only streams at 1 cyc/row when the
# output free dim is >= 256, so recomputing block (3,2) is cheaper than a
# 128-wide f32r matmul.
MVSTART = [0, 128, 256, 256]

_CACHE = {}


def _build_nc():
    from contextlib import ExitStack

    import concourse.bacc as bacc
    import concourse.tile as tile
    from concourse import mybir
    from concourse.masks import make_identity

    f32 = mybir.dt.float32
    f32r = mybir.dt.float32r
    f16 = mybir.dt.float16
    AF = mybir.ActivationFunctionType
    ALU = mybir.AluOpType

    nc = bacc.Bacc(None, target_bir_lowering=False)
    # x stays float32 end-to-end on the load path: the DMA cast unit ROUNDS
    # when the destination dtype is float32r, which would corrupt the
    # residual. Reduced-precision matmul operands are produced by engine
    # cast-copies (ScalarE/VectorE).
    x_d = nc.dram_tensor("x", [BPC, C, N], f32, kind="ExternalInput")
    g_d = nc.dram_tensor("gamma", [1], f32, kind="ExternalInput")
    # s-major out layout: each s-group store writes 4KB contiguous per
    # partition (fp16 needs >=2KB lines for full DMA rate; the natural
    # [C, N] layout would give 1KB lines). Host unscrambles.
    o_d = nc.dram_tensor(
        "out", [BPC, NCH, P, CT, FD], f16, kind="ExternalOutput"
    )

    with ExitStack() as ctx:
        tc = ctx.enter_context(tile.TileContext(nc))
        singles = ctx.enter_context(tc.tile_pool(name="singles", bufs=1))
        bigq = ctx.enter_context(tc.tile_pool(name="bigq", bufs=2))
        qtp = ctx.enter_context(tc.tile_pool(name="qtp", bufs=5))
        qrp = ctx.enter_context(tc.tile_pool(name="qrp", bufs=3))
        mats = ctx.enter_context(tc.tile_pool(name="mats", bufs=4))
        outp = ctx.enter_context(tc.tile_pool(name="outp", bufs=3))
        # ST gets its own 8-deep pool: with a shared 4-buffer ring, batch 1's
        # ST copies would carry a write-after-read dep on batch 0's last
        # deferred mm2 groups (which still read batch 0's ST tiles)
        stp = ctx.enter_context(tc.tile_pool(name="stp", bufs=8))
        smallp = ctx.enter_context(tc.tile_pool(name="small", bufs=8))
        psp = ctx.enter_context(tc.tile_pool(name="ps", bufs=8, space="PSUM"))

        def ps_tile():
            return psp.tile([P, FD], f32, tag="ps", name="ps")

        LOOK = 2

        def emit_load(b, chunk=FD):
            xb = x_d[b].rearrange("(ct p) n -> p ct n", p=P)
            ob = o_d[b]
            q = bigq.tile([P, CT, N], f32, tag="q")
            # NB: 512-col-or-wider chunks only — narrower pieces drop the DMA
            # to sub-2KB lines, which transfer ~2x slower (measured). Batch 0
            # uses 512-col chunks (head latency); batch 1 coarser 1024-col.
            for s in range(N // chunk):
                nc.sync.dma_start(
                    out=q[:, :, s * chunk : (s + 1) * chunk],
                    in_=xb[:, :, s * chunk : (s + 1) * chunk],
                )
            return {"q": q, "xb": xb, "ob": ob}

        def emit_tr(st, k):
            q = st["q"]
            pst = psp.tile([P, FD], f32, tag="ps", name="pstr")
            for t in range(CT):
                nc.tensor.transpose(
                    pst[:, t * P : (t + 1) * P],
                    q[:, t, k * P : (k + 1) * P],
                    ident[:],
                )
            # rounding cast f32 -> f32r makes qk a legal f32r operand
            qk = qtp.tile([P, C], f32r, tag="qt")
            if k % 2 == 0:
                nc.scalar.copy(qk[:], pst[:])
            else:
                nc.vector.tensor_copy(qk[:], pst[:])
            st["qt"][k] = qk

        def emit_mm1(st, k):
            qkr = st["qt"][k]
            psE = st["psE"]
            for t in range(CT):
                w = C - MVSTART[t]
                nc.tensor.matmul(
                    psE[t][:, :w],
                    qkr[:, t * P : (t + 1) * P],
                    qkr[:, MVSTART[t] :],
                    start=(k == 0),
                    stop=(k == KT - 1),
                )

        def emit_cast(st, s):
            q = st["q"]
            qr = qrp.tile([P, CT, FD], f16, tag="qr")
            nc.vector.tensor_copy(qr[:], q[:, :, s * FD : (s + 1) * FD])
            st["qrs"][s] = qr

        def emit_mm2_s(st, s, last=False):
            # one s-chunk of mm2 + epilogue: 4 psU groups, 1 aggregated store
            if s == 0:
                emit_cast(st, 0)
                emit_cast(st, 1)
            if s + 2 < NCH:
                emit_cast(st, s + 2)
            qr = st["qrs"][s]
            q, ob, ST, grz = st["q"], st["ob"], st["ST"], st["grz"]
            ot = outp.tile([P, CT, FD], f16, tag="ot")
            for t in range(CT):
                pu = ps_tile()
                for jt in range(CT):
                    nc.tensor.matmul(
                        pu[:],
                        ST[jt][:, t * P : (t + 1) * P],
                        qr[:, jt, :],
                        start=(jt == 0),
                        stop=(jt == CT - 1),
                    )
                if st["scaled"]:
                    # gamma/Z folded into S: bare residual add
                    nc.vector.tensor_add(
                        ot[:, t, :], pu[:], q[:, t, s * FD : (s + 1) * FD]
                    )
                elif t < 2:
                    nc.scalar.mul(pu[:], pu[:], grz[t][:])
                    nc.vector.tensor_add(
                        ot[:, t, :], pu[:], q[:, t, s * FD : (s + 1) * FD]
                    )
                else:
                    nc.vector.scalar_tensor_tensor(
                        ot[:, t, :],
                        pu[:],
                        grz[t][:],
                        q[:, t, s * FD : (s + 1) * FD],
                        op0=ALU.mult,
                        op1=ALU.add,
                    )
                if last and t == 1:
                    # half-store so the final drain overlaps tiles 2-3
                    nc.sync.dma_start(out=ob[s, :, 0:2, :], in_=ot[:, 0:2, :])
            if last:
                nc.sync.dma_start(out=ob[s, :, 2:CT, :], in_=ot[:, 2:CT, :])
            else:
                nc.sync.dma_start(out=ob[s], in_=ot[:])

        def emit_gram(st, prev, skip_chunks=0):
            """Transposes + Gram matmul for `st`, burst-interleaved with the
            previous batch's attention-apply (mm2) so PE never idles long
            enough for the HAM clock gate to re-throttle."""
            st["psE"] = [ps_tile() for _ in range(CT)]
            if "qt" not in st:
                st["qt"] = [None] * KT
            # chunks below skip_chunks had their transposes emitted during the
            # previous batch's softmax phase; emit their (lookahead-deferred)
            # Gram matmuls now so the accumulation starts with chunk 0
            for kk in range(max(0, skip_chunks - LOOK)):
                emit_mm1(st, kk)
            # chunks processed in groups (GRP x4 transposes, then GRP x4
            # Gram matmuls): transpose-mode <-> regular-mode switches flush
            # the PE pipeline, so longer same-mode runs pipeline better.
            # Pairs everywhere: quads were tried for the solo first-batch
            # phase (4 spare PSUM banks) and measured ~5% WORSE - with all
            # 8 banks live the pool serializes on bank recycling.
            grp = 2
            for k in range(skip_chunks, KT, grp):
                for kk in range(k, k + grp):
                    emit_tr(st, kk)
                if k >= grp:
                    for kk in range(k - grp, k):
                        emit_mm1(st, kk)
                # only 4 of 8 s-groups here: the other four fill this batch's
                # own softmax phase, where the PE would otherwise idle
                if (
                    prev is not None
                    and k >= 6
                    and (k - 6) % 4 == 0
                    and (k - 6) // 4 < NCH - 4
                ):
                    emit_mm2_s(prev, (k - 6) // 4)
            for k in range(KT - grp, KT):
                emit_mm1(st, k)

        def emit_softmax(st, prev=None, scale_s=False):
            # ---- copy E out of PSUM; mirror strictly-lower blocks ----
            psE = st["psE"]
            E = []
            for t in range(CT):
                e = mats.tile([P, FD], f32, tag="E")
                w = C - MVSTART[t]
                if t % 2 == 0:
                    nc.scalar.copy(e[:, MVSTART[t] :], psE[t][:, :w])
                else:
                    nc.vector.tensor_copy(e[:, MVSTART[t] :], psE[t][:, :w])
                E.append(e)
            # E[t][:, s-block] = E[s][:, t-block].T for s < t (exact fp32
            # transposes: E magnitudes are ~4e3 and feed exp directly, so
            # low-precision rounding here would be a real error).
            for t in range(1, CT):
                for s in range(t):
                    if t == 3 and s == 2:
                        continue  # computed directly via the widened row-tile 3
                    pm = ps_tile()
                    nc.tensor.transpose(
                        pm[:, :P], E[s][:, t * P : (t + 1) * P], ident[:]
                    )
                    if (t + s) % 2 == 0:
                        nc.scalar.copy(E[t][:, s * P : (s + 1) * P], pm[:, :P])
                    else:
                        nc.vector.tensor_copy(
                            E[t][:, s * P : (s + 1) * P], pm[:, :P]
                        )

            # deferred mm2 s-group of the previous batch keeps the PE busy
            # while the rowmin/exp chains run on VectorE/ScalarE; for the
            # first batch, the NEXT batch's first transposes fill in instead
            if prev is not None:
                emit_mm2_s(prev, NCH - 4)
            elif st.get("next") is not None:
                emit_tr(st["next"], 0)
                emit_tr(st["next"], 1)

            # ---- suppression softmax: S = exp(rowmin - E), Z = rowsum(S),
            # S written as fp16 (legal fast-weight-load transpose operand) ----
            S = []
            grz = []
            for t in range(CT):
                rm = smallp.tile([P, 1], f32, tag="rm")
                nc.vector.tensor_reduce(
                    rm[:], E[t][:], axis=mybir.AxisListType.X, op=ALU.min
                )
                s_t = mats.tile([P, FD], f16, tag="S")
                z = smallp.tile([P, 1], f32, tag="z")
                nc.scalar.activation(
                    s_t[:], E[t][:], AF.Exp, bias=rm[:], scale=-1.0, accum_out=z[:]
                )
                rz = smallp.tile([P, 1], f32, tag="rz")
                nc.vector.reciprocal(rz[:], z[:])
                g = smallp.tile([P, 1], f32, tag="grz")
                nc.vector.tensor_mul(g[:], rz[:], gam[:])
                if scale_s:
                    # fold gamma/Z into S itself (per-partition row scale):
                    # the attention-apply output is then pre-scaled and the
                    # epilogue is a bare residual add. Only done where the
                    # softmax phase has deferred-mm2 PE cover for the longer
                    # chain (the last batch); the first batch keeps the
                    # scale in its epilogue.
                    nc.scalar.mul(s_t[:], s_t[:], g[:])
                S.append(s_t)
                grz.append(g)

            if prev is not None:
                emit_mm2_s(prev, NCH - 3)
            elif st.get("next") is not None:
                emit_tr(st["next"], 2)
                emit_tr(st["next"], 3)

            # ---- ST = S.T (attention^T), 128x128 fp16 blocks on PE ----
            # jt-major: each pstS[jt] completes after 4 transposes so its
            # SBUF copy overlaps the next jt's transposes; the final deferred
            # mm2 group runs after, covering the last copies' drain so the
            # next phase's first matmul never waits on ST
            ST = []
            for jt in range(CT):
                pstS = psp.tile([P, FD], f16, tag="ps", name="pstS")
                for t in range(CT):
                    nc.tensor.transpose(
                        pstS[:, t * P : (t + 1) * P],
                        S[t][:, jt * P : (jt + 1) * P],
                        identh[:],
                    )
                stj = stp.tile([P, FD], f16, tag="ST")
                if jt % 2 == 0:
                    nc.scalar.copy(stj[:], pstS[:])
                else:
                    nc.vector.tensor_copy(stj[:], pstS[:])
                ST.append(stj)
            if prev is not None:
                emit_mm2_s(prev, NCH - 2)
                emit_mm2_s(prev, NCH - 1)
            st["ST"] = ST
            st["scaled"] = scale_s
            st["grz"] = grz
            st["qrs"] = [None] * NCH

        # ---- pipelined driver: batch b's Gram phase overlaps batch b-1's
        # attention-apply phase on the PE ----
        st0 = emit_load(0)
        st1 = emit_load(1, chunk=2 * FD)

        # dummy transposes ramp the PE while the first load chunk is still
        # in flight, so the HAM clock gate reaches 8/8 before real work
        # starts (a cold PE runs ~3.4us at half clock otherwise). The
        # scratch operand comes from a VectorE memset so warm-up starts
        # ~3us before the (gpsimd-built) identity is ready.
        wsrc = singles.tile([P, P], f32)
        nc.vector.memset(wsrc[:], 0.0)
        for _ in range(8):
            pw = psp.tile([P, P], f32, tag="ps", name="warm")
            for _r in range(4):
                nc.tensor.transpose(pw[:], wsrc[:], wsrc[:])

        ident = singles.tile([P, P], f32)
        make_identity(nc, ident)
        identh = singles.tile([P, P], f16)
        nc.vector.tensor_copy(identh[:], ident[:])

        # gamma broadcast to all partitions as a per-partition scalar
        gam = singles.tile([P, 1], f32)
        nc.gpsimd.dma_start(out=gam[:], in_=g_d[:].to_broadcast([P, 1]))

        emit_gram(st0, None)
        st1["qt"] = [None] * KT
        st0["next"] = st1
        emit_softmax(st0, None)
        emit_gram(st1, st0, skip_chunks=4)
        emit_softmax(st1, st0, scale_s=True)
        for s in range(NCH):
            emit_mm2_s(st1, s, last=(s == NCH - 1))

    nc.compile()
    return nc


def _get_nc():
    if "nc" not in _CACHE:
        _CACHE["nc"] = _build_nc()
    return _CACHE["nc"]


def kernel(x: np.ndarray, gamma: np.ndarray) -> np.ndarray:
    from concourse.bass_utils import run_bass_kernel_spmd

    nc = _get_nc()
    x = np.ascontiguousarray(np.asarray(x, dtype=np.float32))
    gamma = np.ascontiguousarray(np.asarray(gamma, dtype=np.float32))
    xs = x.reshape(B, C, N)
    in_maps = [
        {
            "x": np.ascontiguousarray(xs[c * BPC : (c + 1) * BPC]),
            "gamma": gamma,
        }
        for c in range(N_CORES)
    ]
    res = run_bass_kernel_spmd(nc, in_maps, core_ids=list(range(N_CORES)))
    out = np.stack(
        [np.asarray(res.results[c]["out"]) for c in range(N_CORES)], axis=0
    )
    # [cores, BPC, NCH, P, CT, FD] -> [B, C=(ct*128+p), N=(s*512+f)]
    out = out.reshape(N_CORES * BPC, NCH, P, CT, FD)
    out = out.transpose(0, 3, 2, 1, 4).reshape(B, C, N)
    return out.reshape(B, C, H, W).astype(np.float32)
